# revision 25
# baseline (speedup 1.0000x reference)
"""Two-layer GAT on 8 Trainium2 NeuronCores — v2 (engine-balanced, bf16).

Structure (per core; rows of the dense NxN attention matrix sharded):
  Phase A: h1 = x @ W1ext built in bf16 (PE), alpha columns kept fp32.
           asrow (alpha_src row) via small matmuls from xTown; broadcast
           rows built with PE outer products.
  Phase B: single fused 4-head layer-1 sweep over the 64 t-tiles.
           Heads 0,1 ("scalar path"): one patched-table ACT instruction
           computes exp(leaky_relu(as+at)) per tile; DVE applies the mask.
           Heads 2,3 ("factored path"): softmax columns are rescaled by
           e^{-0.2 as} (cancels in the normalization), which turns the
           edge weight into max(e^{0.8as}·e^{at}, e^{0.2at}) — one DVE
           tensor_scalar (4x mode) + one mask multiply (2x mode) per tile.
           All matmuls bf16 into 4 persistent PSUM accumulators.
  Phase C: normalize+ELU -> hl2T (bf16), layer-2 payload build, bf16
           AllGather of [h2|1|at2|as2] rows.
  Phase D: layer-2 sweep, all j factored-path. Dup corrections exact.
  Phase E: transpose + log_softmax epilogue (fp32), as v1.

Duplicate edges are excluded from the host-built mask and corrected
exactly with indirect-DMA gathers + rank-limited matmuls (as v1), with
an extra e^{-0.2 as} factor on factored heads to match the column scale.
"""

import math
from dataclasses import dataclass

import ml_dtypes
import numpy as np

import concourse.bass as bass
import concourse.mybir as mybir
import concourse.tile as tile
from concourse import bacc
from concourse.bass_utils import run_bass_kernel_spmd
from concourse.masks import make_identity

F32 = mybir.dt.float32
F32R = mybir.dt.float32r
BF16 = mybir.dt.bfloat16
FP16 = mybir.dt.float16
I32 = mybir.dt.int32
AF = mybir.ActivationFunctionType
OP = mybir.AluOpType
AX = mybir.AxisListType

P = 128

# engine-assignment knobs
GP_MASK_L1 = False   # run head-0's mask multiply on gpsimd
GP_MASK_L2 = 0       # every k-th L2 mask multiply on gpsimd (0 = off)


@dataclass(frozen=True)
class GATConfig:
    n: int = 8192
    f_in: int = 512
    nhid: int = 64
    heads: int = 4
    nclass: int = 16
    ncores: int = 8
    dup_rounds: int = 1

    @property
    def s(self):
        return self.n // self.ncores

    @property
    def jt(self):
        return self.n // P

    @property
    def sc(self):
        return self.s // P

    @property
    def kt1(self):
        return self.f_in // P

    @property
    def fcat(self):
        return self.nhid * self.heads

    @property
    def kt2(self):
        return self.fcat // P

    @property
    def hs1(self):          # per-head column stride in h1buf: h(64)|one|spare
        return self.nhid + 2

    @property
    def w1cols(self):
        return self.hs1 * self.heads

    @property
    def w2cols(self):       # W2^T(16) | va2_t | va2_s
        return self.nclass + 2

    @property
    def payc(self):         # h2(16) | one | at2hi | at2lo | as2hi | as2lo | pad
        return self.nclass + 6


def _ts(i, sz):
    return slice(i * sz, (i + 1) * sz)


def _halves(s):
    return [slice(h0, min(h0 + 512, s)) for h0 in range(0, s, 512)]


SC_HEADS = (0, 1)
FC_HEADS = (2, 3)


def build_gat_kernel(tc, cfg: GATConfig, io):
    nc = tc.nc
    n, s, jt = cfg.n, cfg.s, cfg.jt
    hs, nh, ncl = cfg.hs1, cfg.nhid, cfg.nclass
    MCH = 2
    njc = jt // MCH
    R = cfg.dup_rounds

    xT, xTown = io["xT"], io["xTown"]
    w1t, w2t, maskT = io["w1t"], io["w2t"], io["maskT"]
    tdup, sdup, dmult, sonehot = io["tdup"], io["sdup"], io["dmult"], io["sonehot"]
    out_dram = io["out"]

    with (
        tc.tile_pool(name="persist", bufs=1) as sb1,
        tc.tile_pool(name="stream", bufs=2) as sbs,
        tc.tile_pool(name="misc", bufs=2) as sbm,
        tc.tile_pool(name="dram", bufs=1, space="DRAM") as dram,
    ):
        # ---------------- persistent tiles ------------------------------
        w1t_sb = sb1.tile([P, cfg.kt1, cfg.w1cols], FP16, tag="w1t_sb")
        nc.sync.dma_start(w1t_sb[:], w1t[:].rearrange("(k p) c -> p k c", p=P))
        w2t_sb = sb1.tile([P, cfg.kt2, cfg.w2cols], F32R, tag="w2t_sb")
        nc.sync.dma_start(w2t_sb[:], w2t[:].rearrange("(k p) c -> p k c", p=P))

        h1buf = sb1.tile([P, jt, cfg.w1cols], FP16, tag="h1buf")
        atcols = sb1.tile([P, jt, cfg.heads, 2], F32, tag="atcols")
        hl2T = sb1.tile([P, cfg.kt2, s], F32R, tag="hl2T")
        g_all = sb1.tile([P, jt, cfg.payc], FP16, tag="g_all")
        pay_all = sb1.tile([P, cfg.sc, cfg.payc], FP16, tag="pay_all")

        ones1 = sb1.tile([1, P], F32R, tag="ones1")
        nc.vector.memset(ones1[:].bitcast(F32), 1.0)
        # sel_all[:, h*P:(h+1)*P] is a [4,128] selector whose row h is ones;
        # matmul(sel_h, row4) broadcasts row h of a [4,s] tile to 128 parts.
        sel_all = sb1.tile([cfg.heads, cfg.heads * P], F32R, tag="sel_all")
        nc.sync.dma_start(sel_all[:], io["sel"][:])
        sel16 = sb1.tile([cfg.heads, cfg.heads * P], FP16, tag="sel16")
        nc.vector.tensor_copy(sel16[:], sel_all[:])
        ones16 = sb1.tile([1, P], FP16, tag="ones16")
        nc.vector.memset(ones16[:], 1.0)
        ident = sb1.tile([P, P], F32, tag="ident")
        make_identity(nc, ident[:])

        # ones columns of h1buf (slot nh of each head's stride)
        for h in range(cfg.heads):
            nc.vector.memset(h1buf[:, :, h * hs + nh], 1.0)
            nc.vector.memset(h1buf[:, :, h * hs + nh + 1], 0.0)

        tdup_sb = sb1.tile([P, R, 1], I32, tag="tdup_sb")
        sdup_sb = sb1.tile([P, R, 1], I32, tag="sdup_sb")
        dmult_sb = sb1.tile([P, R, 1], F32, tag="dmult_sb")
        nc.sync.dma_start(tdup_sb[:], tdup[:].rearrange("(r p) c -> p r c", p=P))
        nc.sync.dma_start(sdup_sb[:], sdup[:].rearrange("(r p) c -> p r c", p=P))
        nc.sync.dma_start(dmult_sb[:], dmult[:].rearrange("(r p) c -> p r c", p=P))
        soh_sb = sb1.tile([P, R, s], BF16, tag="soh_sb")
        nc.sync.dma_start(soh_sb[:], sonehot[:].rearrange("(r p) c -> p r c", p=P))

        if "h1dbg" in io:
            h1dram = io["h1dbg"]
            aldram = io["aldbg"]
        else:
            h1dram = dram.tile([n, cfg.w1cols], FP16)
            aldram = dram.tile([n, 2 * cfg.heads], F32)
        pay_dram = dram.tile([s, cfg.payc], FP16)
        g_dram = dram.tile([n, cfg.payc], FP16)

        # per-head factored-attention tiles. Column scale for column s is
        # c_s = e^{-as - ATM + 9} (ATM = global max at per head), making
        # branch A = e^{at - ATM + 9} (per-t constant, <= e^9) and branch
        # B = e^{-0.8as} * e^{0.2at - ATM + 9} -- all fp16-safe.
        rhob = {}     # e^{-0.8 as} broadcast rows, fp16
        ea9 = {}      # e^{at - ATM + 9} per (t-partition, j), fp32
        eb9 = {}      # e^{0.2 at - ATM + 9}, fp32
        c9 = {}       # 9 - ATM, [P, 1] fp32
        for h in range(cfg.heads):
            rhob[h] = sb1.tile([P, s], FP16, tag=f"rhob{h}", name=f"rhob{h}")
            ea9[h] = sb1.tile([P, jt], F32, tag=f"ea9{h}", name=f"ea9{h}")
            eb9[h] = sb1.tile([P, jt], F32, tag=f"eb9{h}", name=f"eb9{h}")
            c9[h] = sb1.tile([P, 1], F32, tag=f"c9{h}", name=f"c9{h}")

        # =========== Phase A: h1 build + rows/broadcast setup ===========
        with tc.tile_pool(name="psA", bufs=1, space="PSUM") as psA_pool:
            # A1: asrow4[h, :] = alpha_src row per head (from own x block)
            psrow = psA_pool.tile([cfg.heads, s], F32, tag="psrow")
            for k in range(cfg.kt1):
                xo = sbs.tile([P, s], FP16, tag="xo")
                nc.sync.dma_start(xo[:], xTown[_ts(k, P), :])
                ascols = w1t_sb[:, k, :].rearrange(
                    "p (h c) -> p h c", c=hs)[:, :, nh:nh + 1]
                for hsl in _halves(s):
                    nc.tensor.matmul(psrow[:, hsl], ascols, xo[:, hsl],
                                     start=(k == 0), stop=(k == cfg.kt1 - 1))
            asrow4 = sb1.tile([cfg.heads, s], F32R, tag="asrow4")
            nc.scalar.copy(asrow4[:], psrow[:])

            # A2: rhorow = e^{-0.8 as} (true exp via pos/neg split)
            t08 = sbm.tile([cfg.heads, s], F32, tag="rowA", bufs=1, name="t08")
            nc.vector.tensor_scalar_mul(t08[:], asrow4[:], -0.8)
            tpos = sbm.tile([cfg.heads, s], F32, tag="rowB", bufs=1, name="tpos")
            nc.vector.tensor_single_scalar(tpos[:], t08[:], 0.0, OP.max)
            tneg = sbm.tile([cfg.heads, s], F32, tag="rowC", bufs=1, name="tneg")
            nc.vector.tensor_single_scalar(tneg[:], t08[:], 0.0, OP.min)
            epos = sbm.tile([cfg.heads, s], F32, tag="rowD", bufs=1, name="epos")
            nc.scalar.activation(epos[:], tpos[:], AF.Exp)
            eneg = sbm.tile([cfg.heads, s], F32, tag="rowE", bufs=1, name="eneg")
            nc.scalar.activation(eneg[:], tneg[:], AF.Exp, scale=5.0)
            rhorow = sb1.tile([cfg.heads, s], F32R, tag="rhorow")
            nc.vector.tensor_mul(rhorow[:], epos[:], eneg[:])

            # A3: PE outer-product broadcasts of the rho rows (fp16)
            for h in range(cfg.heads):
                psb = psA_pool.tile([P, s], F32, tag="psb", name=f"psbB{h}")
                for hsl in _halves(s):
                    nc.tensor.matmul(psb[:, hsl], sel_all[:, _ts(h, P)],
                                     rhorow[:, hsl], start=True, stop=True)
                nc.scalar.copy(rhob[h][:], psb[:])

            # A5: h1 build over all 64 j-tiles
            for j in range(jt):
                xtc = sbs.tile([P, cfg.kt1, P], FP16, tag="xtc", bufs=3)
                nc.sync.dma_start(
                    xtc[:], xT[:, _ts(j, P)].rearrange("(k p) m -> p k m", p=P))
                psA = psA_pool.tile([P, cfg.w1cols], F32, tag="psA", bufs=2)
                for k in range(cfg.kt1):
                    nc.tensor.matmul(psA[:], xtc[:, k, :], w1t_sb[:, k, :],
                                     start=(k == 0), stop=(k == cfg.kt1 - 1))
                psA_h = psA[:].rearrange("p (h c) -> p h c", c=hs)
                h1v = h1buf[:, j, :].rearrange("p (h c) -> p h c", c=hs)
                nc.scalar.copy(h1v[:, :, 0:nh], psA_h[:, :, 0:nh])
                nc.vector.tensor_copy(atcols[:, j], psA_h[:, :, nh:nh + 2])
                nc.gpsimd.dma_start(h1dram[_ts(j, P), :], h1buf[:, j, :])
                nc.gpsimd.dma_start(
                    aldram[_ts(j, P), :].rearrange("p (h c) -> p h c", c=2),
                    atcols[:, j])

            # A6: per-head global at-max, then ea9/eb9 tiles
            atm4 = sbm.tile([P, cfg.heads], F32, tag="atm4", bufs=1, name="atm4")
            for h in range(cfg.heads):
                nc.vector.tensor_reduce(atm4[:, h:h + 1], atcols[:, :, h, 1],
                                        axis=AX.X, op=OP.max)
            ptr = psA_pool.tile([cfg.heads, P], F32, tag="ptr")
            nc.tensor.transpose(ptr[:], atm4[:], ident[:])
            tr4 = sbm.tile([cfg.heads, P], F32, tag="tr4", bufs=1, name="tr4")
            nc.scalar.copy(tr4[:], ptr[:])
            amax4 = sbm.tile([cfg.heads, 1], FP16, tag="amax4", bufs=1,
                             name="amax4")
            nc.vector.tensor_reduce(amax4[:], tr4[:], axis=AX.X, op=OP.max)
            for h in range(cfg.heads):
                psc = psA_pool.tile([P, 1], F32, tag="psc", name=f"psc{h}")
                nc.tensor.matmul(psc[:], sel16[:, _ts(h, P)], amax4[:],
                                 start=True, stop=True)
                atmb = sbm.tile([P, 1], F32, tag=f"atmb{h}", bufs=1,
                                name=f"atmb{h}")
                nc.scalar.copy(atmb[:], psc[:])
                nc.vector.tensor_scalar(out=c9[h][:], in0=atmb[:],
                                        scalar1=-1.0, scalar2=9.0,
                                        op0=OP.mult, op1=OP.add)
                at_h = atcols[:, :, h, 1]              # [P, jt]
                t9 = sbm.tile([P, jt], F32, tag="attmp", name="t9")
                nc.vector.tensor_scalar(out=t9[:], in0=at_h, scalar1=atmb[:],
                                        scalar2=9.0, op0=OP.subtract,
                                        op1=OP.add)
                rp = sbm.tile([P, jt], F32, tag="attmp3", name="rp")
                nc.vector.tensor_single_scalar(rp[:], t9[:], 0.0, OP.max)
                rn = sbm.tile([P, jt], F32, tag="attmp4", name="rn")
                nc.vector.tensor_single_scalar(rn[:], t9[:], 0.0, OP.min)
                e1 = sbm.tile([P, jt], F32, tag="attmp2", name="e1")
                nc.scalar.activation(e1[:], rp[:], AF.Exp)
                e2 = sbm.tile([P, jt], F32, tag="attmp2", name="e2")
                nc.scalar.activation(e2[:], rn[:], AF.Exp, scale=5.0)
                nc.vector.tensor_mul(ea9[h][:], e1[:], e2[:])
                t9b = sbm.tile([P, jt], F32, tag="attmp", name="t9b")
                nc.vector.tensor_scalar(out=t9b[:], in0=at_h, scalar1=0.2,
                                        scalar2=c9[h][:], op0=OP.mult,
                                        op1=OP.add)
                rpb = sbm.tile([P, jt], F32, tag="attmp3", name="rpb")
                nc.vector.tensor_single_scalar(rpb[:], t9b[:], 0.0, OP.max)
                rnb = sbm.tile([P, jt], F32, tag="attmp4", name="rnb")
                nc.vector.tensor_single_scalar(rnb[:], t9b[:], 0.0, OP.min)
                e3 = sbm.tile([P, jt], F32, tag="attmp2", name="e3")
                nc.scalar.activation(e3[:], rpb[:], AF.Exp)
                e4 = sbm.tile([P, jt], F32, tag="attmp2", name="e4")
                nc.scalar.activation(e4[:], rnb[:], AF.Exp, scale=5.0)
                nc.vector.tensor_mul(eb9[h][:], e3[:], e4[:])

            # A7: dup gathers (h1dram/aldram complete)
            hdup1, alt1, als1 = [], [], []
            for r in range(R):
                ht = sb1.tile([P, cfg.w1cols], FP16, tag=f"hdup1_{r}",
                              name=f"hdup1_{r}")
                nc.gpsimd.indirect_dma_start(
                    out=ht[:], out_offset=None, in_=h1dram[:],
                    in_offset=bass.IndirectOffsetOnAxis(
                        ap=tdup_sb[:, r, :], axis=0))
                hdup1.append(ht)
                at_ = sb1.tile([P, 2 * cfg.heads], F32, tag=f"alt1_{r}",
                               name=f"alt1_{r}")
                nc.gpsimd.indirect_dma_start(
                    out=at_[:], out_offset=None, in_=aldram[:],
                    in_offset=bass.IndirectOffsetOnAxis(
                        ap=tdup_sb[:, r, :], axis=0))
                alt1.append(at_)
                as_ = sb1.tile([P, 2 * cfg.heads], F32, tag=f"als1_{r}",
                               name=f"als1_{r}")
                nc.gpsimd.indirect_dma_start(
                    out=as_[:], out_offset=None, in_=aldram[:],
                    in_offset=bass.IndirectOffsetOnAxis(
                        ap=sdup_sb[:, r, :], axis=0))
                als1.append(as_)

        # helpers for dup-correction math --------------------------------
        def dup_weight(u, r, as_d, c9_ap):
            """dl = exp(dmult * leaky_relu(u) - as_d + (9 - ATM))"""
            w8 = sbm.tile([P, 1], F32, tag="dup_w8", name="w8")
            nc.vector.tensor_scalar(out=w8[:], in0=u[:], scalar1=0.0,
                                    scalar2=0.8, op0=OP.max, op1=OP.mult)
            w_ = sbm.tile([P, 1], F32, tag="dup_w_", name="w_")
            nc.vector.scalar_tensor_tensor(out=w_[:], in0=u[:], scalar=0.2,
                                           in1=w8[:], op0=OP.mult, op1=OP.add)
            nc.vector.tensor_mul(w_[:], w_[:], dmult_sb[:, r, :])
            wq = sbm.tile([P, 1], F32, tag="dup_wq", name="wq")
            nc.vector.tensor_scalar(out=wq[:], in0=w_[:], scalar1=as_d,
                                    scalar2=c9_ap, op0=OP.subtract, op1=OP.add)
            wp = sbm.tile([P, 1], F32, tag="dup_wp", name="wp")
            nc.vector.tensor_single_scalar(wp[:], wq[:], 0.0, OP.max)
            wn = sbm.tile([P, 1], F32, tag="dup_wn", name="wn")
            nc.vector.tensor_single_scalar(wn[:], wq[:], 0.0, OP.min)
            dp = sbm.tile([P, 1], F32, tag="dup_dp", name="dp")
            nc.scalar.activation(dp[:], wp[:], AF.Exp)
            dn = sbm.tile([P, 1], F32, tag="dup_dn", name="dn")
            nc.scalar.activation(dn[:], wn[:], AF.Exp, scale=5.0)
            dl = sbm.tile([P, 1], F32, tag="dup_dl", name="dl")
            nc.vector.tensor_mul(dl[:], dp[:], dn[:])
            return dl

        # =========== Phase B: fused 4-head layer-1 sweep ================
        otT = []
        with tc.tile_pool(name="psB", bufs=1, space="PSUM") as psB_pool:
            ps_att = [psB_pool.tile([nh + 1, s], F32, tag=f"att{h}",
                                    name=f"att{h}")
                      for h in range(cfg.heads)]
            for jc in range(njc):
                mtile = sbs.tile([P, MCH, s], FP16, tag="mtile")
                nc.sync.dma_start(
                    mtile[:],
                    maskT[_ts(jc, MCH * P), :].rearrange(
                        "(c p) ss -> p c ss", p=P))
                for jj in range(MCH):
                    j = jc * MCH + jj
                    ptp = sbs.tile([P, cfg.heads, s], FP16, tag="ptp")
                    for h in range(cfg.heads):
                        v = sbs.tile([P, s], FP16, tag="vfac", bufs=4,
                                     name=f"v{h}")
                        nc.vector.tensor_scalar(
                            out=v[:], in0=rhob[h][:],
                            scalar1=eb9[h][:, j:j + 1],
                            scalar2=ea9[h][:, j:j + 1],
                            op0=OP.mult, op1=OP.max)
                        eng = nc.gpsimd if (GP_MASK_L1 and h == 0) else nc.vector
                        eng.tensor_mul(ptp[:, h, :], v[:], mtile[:, jj, :])
                    for h in range(cfg.heads):
                        lhs = h1buf[:, j, h * hs: h * hs + nh + 1]
                        for hsl in _halves(s):
                            nc.tensor.matmul(ps_att[h][:, hsl], lhs,
                                             ptp[:, h, hsl],
                                             start=(j == 0), stop=False)
            # dup corrections close each head's accumulation
            for h in range(cfg.heads):
                for r in range(R):
                    u = sbm.tile([P, 1], F32, tag="dup_u", name="u")
                    nc.vector.tensor_add(u[:], alt1[r][:, 2 * h + 1:2 * h + 2],
                                         als1[r][:, 2 * h:2 * h + 1])
                    dl = dup_weight(u, r, als1[r][:, 2 * h:2 * h + 1],
                                    c9[h][:])
                    hf = sbm.tile([P, nh + 1], F32, tag="hscf", name="hf")
                    nc.vector.tensor_scalar_mul(
                        hf[:], hdup1[r][:, h * hs: h * hs + nh + 1], dl[:])
                    hsc = sbm.tile([P, nh + 1], BF16, tag="hsc", name="hsc")
                    nc.vector.tensor_copy(hsc[:], hf[:])
                    hlo = sbm.tile([P, nh + 1], BF16, tag="hsclo", name="hlo")
                    nc.vector.tensor_sub(hlo[:], hf[:], hsc[:])
                    for part in (hsc, hlo):
                        for hsl in _halves(s):
                            nc.tensor.matmul(
                                ps_att[h][:, hsl], part[:], soh_sb[:, r, hsl],
                                start=False,
                                stop=(r == R - 1 and part is hlo))
            # evacuate psums (split across scalar/vector engines)
            for h in range(cfg.heads):
                ot = sb1.tile([nh + 1, s], F32, tag=f"otT{h}", name=f"otT{h}")
                if h % 2 == 0:
                    nc.scalar.copy(ot[:], ps_att[h][:])
                else:
                    nc.vector.tensor_copy(ot[:], ps_att[h][:])
                otT.append(ot)

        if "otdbg" in io:
            for h in range(cfg.heads):
                nc.gpsimd.dma_start(io["otdbg"][_ts(h, nh + 1), :], otT[h][:])
        # =========== Phase C: normalize + ELU + payload + AllGather =====
        with tc.tile_pool(name="psC", bufs=1, space="PSUM") as psC_pool:
            for h in range(cfg.heads):
                rz = sbm.tile([1, s], F32R, tag="rz", bufs=1, name="rz")
                with nc.allow_low_precision(reason="fp22 recip row"):
                    nc.vector.reciprocal(rz[:], otT[h][nh:nh + 1, :])
                rzb = psC_pool.tile([nh, s], F32, tag="rzb", name=f"rzb{h}")
                for hsl in _halves(s):
                    nc.tensor.matmul(rzb[:, hsl], ones1[:, :nh], rz[0:1, hsl],
                                     start=True, stop=True)
                xn = sbm.tile([nh, s], F32, tag="elu1", bufs=1, name="xn")
                nc.vector.tensor_mul(xn[:], otT[h][:nh, :], rzb[:])
                t1 = sbm.tile([nh, s], F32, tag="elu2", bufs=1, name="t1")
                nc.vector.tensor_single_scalar(t1[:], xn[:], 0.0, OP.min)
                t2 = sbm.tile([nh, s], F32, tag="elu3", bufs=1, name="t2")
                nc.scalar.activation(t2[:], t1[:], AF.Exp, scale=5.0)
                t4 = sbm.tile([nh, s], F32, tag="elu4", bufs=1, name="t4")
                nc.vector.tensor_scalar(out=t4[:], in0=xn[:], scalar1=0.0,
                                        scalar2=-1.0, op0=OP.max, op1=OP.add)
                nc.vector.tensor_add(hl2T[nh * (h % 2):nh * (h % 2) + nh,
                                          h // 2, :], t4[:], t2[:])

            # alpha2_src row + its exp/broadcast
            psr2 = psC_pool.tile([1, s], F32, tag="psr2")
            for k in range(cfg.kt2):
                for hsl in _halves(s):
                    nc.tensor.matmul(psr2[:, hsl], w2t_sb[:, k, ncl + 1:ncl + 2],
                                     hl2T[:, k, hsl],
                                     start=(k == 0), stop=(k == cfg.kt2 - 1))
            a2s = sb1.tile([1, s], F32R, tag="a2s")
            nc.scalar.copy(a2s[:], psr2[:])
            t082 = sbm.tile([1, s], F32, tag="rowA", bufs=1, name="t082")
            nc.vector.tensor_scalar_mul(t082[:], a2s[:], -0.8)
            tp2 = sbm.tile([1, s], F32, tag="rowB", bufs=1, name="tp2")
            nc.vector.tensor_single_scalar(tp2[:], t082[:], 0.0, OP.max)
            tn2 = sbm.tile([1, s], F32, tag="rowC", bufs=1, name="tn2")
            nc.vector.tensor_single_scalar(tn2[:], t082[:], 0.0, OP.min)
            ep2 = sbm.tile([1, s], F32, tag="rowD", bufs=1, name="ep2")
            nc.scalar.activation(ep2[:], tp2[:], AF.Exp)
            en2 = sbm.tile([1, s], F32, tag="rowE", bufs=1, name="en2")
            nc.scalar.activation(en2[:], tn2[:], AF.Exp, scale=5.0)
            r2row = sb1.tile([1, s], F32R, tag="r2row")
            nc.vector.tensor_mul(r2row[:], ep2[:], en2[:])
            psb2 = psC_pool.tile([P, s], F32, tag="psb2")
            for hsl in _halves(s):
                nc.tensor.matmul(psb2[:, hsl], ones1[:], r2row[0:1, hsl],
                                 start=True, stop=True)
            r2b = sb1.tile([P, s], FP16, tag="r2b")
            nc.scalar.copy(r2b[:], psb2[:])

            # payload rows [h2|1|at2|as2]
            nc.vector.memset(pay_all[:, :, ncl], 1.0)
            nc.vector.memset(pay_all[:, :, cfg.payc - 1], 0.0)
            for c in range(cfg.sc):
                ps2 = psC_pool.tile([P, cfg.w2cols], F32, tag="ps2", bufs=2)
                for k in range(cfg.kt2):
                    nc.tensor.matmul(ps2[:], hl2T[:, k, _ts(c, P)],
                                     w2t_sb[:, k, :],
                                     start=(k == 0), stop=(k == cfg.kt2 - 1))
                nc.scalar.copy(pay_all[:, c, 0:ncl], ps2[:, 0:ncl])
                # at2 crosses the gather as an fp16 hi/lo pair (exact sum)
                nc.scalar.copy(pay_all[:, c, ncl + 1:ncl + 2],
                               ps2[:, ncl:ncl + 1])
                nc.vector.tensor_sub(
                    pay_all[:, c, ncl + 2:ncl + 3], ps2[:, ncl:ncl + 1],
                    pay_all[:, c, ncl + 1:ncl + 2])
                nc.scalar.copy(pay_all[:, c, ncl + 3:ncl + 4],
                               ps2[:, ncl + 1:ncl + 2])
                nc.vector.tensor_sub(
                    pay_all[:, c, ncl + 4:ncl + 5], ps2[:, ncl + 1:ncl + 2],
                    pay_all[:, c, ncl + 3:ncl + 4])
                nc.sync.dma_start(pay_dram[_ts(c, P), :], pay_all[:, c, :])

            if "hl2dbg" in io:
                nc.gpsimd.dma_start(
                    io["hl2dbg"][:].rearrange("(k p) c -> p k c", p=P),
                    hl2T[:])
            nc.gpsimd.collective_compute(
                "AllGather", OP.bypass,
                replica_groups=[list(range(cfg.ncores))],
                ins=[pay_dram.opt()], outs=[g_dram.opt()])
            nc.sync.dma_start(
                g_all[:], g_dram[:].rearrange("(j p) c -> p j c", p=P))
            if "gdbg" in io:
                nc.gpsimd.dma_start(io["gdbg"][:], g_dram[:])
                nc.gpsimd.dma_start(io["paydbg"][:], pay_dram[:])

            # layer-2 dup gathers
            hdup2, sdup2 = [], []
            for r in range(R):
                ht = sb1.tile([P, cfg.payc], FP16, tag=f"hdup2_{r}",
                              name=f"hdup2_{r}")
                nc.gpsimd.indirect_dma_start(
                    out=ht[:], out_offset=None, in_=g_dram[:],
                    in_offset=bass.IndirectOffsetOnAxis(
                        ap=tdup_sb[:, r, :], axis=0))
                hdup2.append(ht)
                hs_ = sb1.tile([P, cfg.payc], FP16, tag=f"sdup2_{r}",
                               name=f"sdup2_{r}")
                nc.gpsimd.indirect_dma_start(
                    out=hs_[:], out_offset=None, in_=g_dram[:],
                    in_offset=bass.IndirectOffsetOnAxis(
                        ap=sdup_sb[:, r, :], axis=0))
                sdup2.append(hs_)

        # =========== Phase D: layer-2 sweep (all factored) ==============
        with tc.tile_pool(name="psD", bufs=1, space="PSUM") as psD_pool:
            # ATM2 + e^{at2 - ATM2 + 9}, e^{0.2 at2 - ATM2 + 9}
            at2all = sb1.tile([P, jt], F32, tag="at2all")
            nc.vector.tensor_add(at2all[:], g_all[:, :, ncl + 1],
                                 g_all[:, :, ncl + 2])
            atm2c = sbm.tile([P, 1], F32, tag="atm2c", bufs=1, name="atm2c")
            nc.vector.tensor_reduce(atm2c[:], at2all[:], axis=AX.X, op=OP.max)
            ptr2 = psD_pool.tile([1, P], F32, tag="ptr2")
            nc.tensor.transpose(ptr2[:], atm2c[:], ident[:])
            tr2 = sbm.tile([1, P], F32, tag="tr2", bufs=1, name="tr2")
            nc.scalar.copy(tr2[:], ptr2[:])
            amax2 = sbm.tile([1, 1], FP16, tag="amax2", bufs=1, name="amax2")
            nc.vector.tensor_reduce(amax2[:], tr2[:], axis=AX.X, op=OP.max)
            psc2 = psD_pool.tile([P, 1], F32, tag="psc2")
            nc.tensor.matmul(psc2[:], ones16[:], amax2[:], start=True, stop=True)
            atmb2 = sbm.tile([P, 1], F32, tag="atmb2", bufs=1, name="atmb2")
            nc.scalar.copy(atmb2[:], psc2[:])
            c92 = sb1.tile([P, 1], F32, tag="c92")
            nc.vector.tensor_scalar(out=c92[:], in0=atmb2[:], scalar1=-1.0,
                                    scalar2=9.0, op0=OP.mult, op1=OP.add)
            eat2 = sb1.tile([P, jt], F32, tag="eat2")
            t92 = sbm.tile([P, jt], F32, tag="attmp", name="t92")
            nc.vector.tensor_scalar(out=t92[:], in0=at2all[:], scalar1=atmb2[:],
                                    scalar2=9.0, op0=OP.subtract, op1=OP.add)
            rp2 = sbm.tile([P, jt], F32, tag="attmp3", name="rp2")
            nc.vector.tensor_single_scalar(rp2[:], t92[:], 0.0, OP.max)
            rn2 = sbm.tile([P, jt], F32, tag="attmp4", name="rn2")
            nc.vector.tensor_single_scalar(rn2[:], t92[:], 0.0, OP.min)
            e1 = sbm.tile([P, jt], F32, tag="attmp2", name="e1b")
            nc.scalar.activation(e1[:], rp2[:], AF.Exp)
            e2 = sbm.tile([P, jt], F32, tag="attmp2", name="e2b")
            nc.scalar.activation(e2[:], rn2[:], AF.Exp, scale=5.0)
            nc.vector.tensor_mul(eat2[:], e1[:], e2[:])
            ebt2 = sb1.tile([P, jt], F32, tag="ebt2")
            t92b = sbm.tile([P, jt], F32, tag="attmp", name="t92b")
            nc.vector.tensor_scalar(out=t92b[:], in0=at2all[:], scalar1=0.2,
                                    scalar2=c92[:], op0=OP.mult, op1=OP.add)
            rp2b = sbm.tile([P, jt], F32, tag="attmp3", name="rp2b")
            nc.vector.tensor_single_scalar(rp2b[:], t92b[:], 0.0, OP.max)
            rn2b = sbm.tile([P, jt], F32, tag="attmp4", name="rn2b")
            nc.vector.tensor_single_scalar(rn2b[:], t92b[:], 0.0, OP.min)
            e3 = sbm.tile([P, jt], F32, tag="attmp2", name="e3b")
            nc.scalar.activation(e3[:], rp2b[:], AF.Exp)
            e4 = sbm.tile([P, jt], F32, tag="attmp2", name="e4b")
            nc.scalar.activation(e4[:], rn2b[:], AF.Exp, scale=5.0)
            nc.vector.tensor_mul(ebt2[:], e3[:], e4[:])

            ps2t = psD_pool.tile([ncl + 1, s], F32, tag="att2")
            for jc in range(njc):
                mtile = sbs.tile([P, MCH, s], FP16, tag="mtile")
                nc.sync.dma_start(
                    mtile[:],
                    maskT[_ts(jc, MCH * P), :].rearrange(
                        "(c p) ss -> p c ss", p=P))
                for jj in range(MCH):
                    j = jc * MCH + jj
                    v = sbs.tile([P, s], FP16, tag="vfac", bufs=4, name="v2")
                    nc.vector.tensor_scalar(
                        out=v[:], in0=r2b[:],
                        scalar1=ebt2[:, j:j + 1], scalar2=eat2[:, j:j + 1],
                        op0=OP.mult, op1=OP.max)
                    ptp2 = sbs.tile([P, s], FP16, tag="ptp2", bufs=3)
                    eng = (nc.gpsimd if (GP_MASK_L2 and j % GP_MASK_L2 == 0)
                           else nc.vector)
                    eng.tensor_mul(ptp2[:], v[:], mtile[:, jj, :])
                    for hsl in _halves(s):
                        nc.tensor.matmul(ps2t[:, hsl], g_all[:, j, 0:ncl + 1],
                                         ptp2[:, hsl],
                                         start=(j == 0), stop=False)
            for r in range(R):
                asd2 = sbm.tile([P, 1], F32, tag="dup_asd2", name="asd2")
                nc.vector.tensor_add(asd2[:], sdup2[r][:, ncl + 3:ncl + 4],
                                     sdup2[r][:, ncl + 4:ncl + 5])
                u = sbm.tile([P, 1], F32, tag="dup_u", name="u2")
                nc.vector.tensor_add(u[:], hdup2[r][:, ncl + 1:ncl + 2],
                                     hdup2[r][:, ncl + 2:ncl + 3])
                nc.vector.tensor_add(u[:], u[:], asd2[:])
                dl = dup_weight(u, r, asd2[:], c92[:])
                hf2 = sbm.tile([P, ncl + 1], F32, tag="hscf", name="hf2")
                nc.vector.tensor_scalar_mul(hf2[:], hdup2[r][:, 0:ncl + 1],
                                            dl[:])
                hsc = sbm.tile([P, ncl + 1], BF16, tag="hsc", name="hsc2")
                nc.vector.tensor_copy(hsc[:], hf2[:])
                hlo2 = sbm.tile([P, ncl + 1], BF16, tag="hsclo", name="hlo2")
                nc.vector.tensor_sub(hlo2[:], hf2[:], hsc[:])
                for part in (hsc, hlo2):
                    for hsl in _halves(s):
                        nc.tensor.matmul(ps2t[:, hsl], part[:],
                                         soh_sb[:, r, hsl], start=False,
                                         stop=(r == R - 1 and part is hlo2))

            otT2 = sb1.tile([ncl + 1, s], F32, tag="otT2")
            nc.scalar.copy(otT2[:], ps2t[:])

            # ======= Phase E: transpose + log_softmax epilogue ==========
            for c in range(cfg.sc):
                pst = psD_pool.tile([P, ncl + 1], F32, tag="pst", bufs=2)
                nc.tensor.transpose(pst[:], otT2[:, _ts(c, P)],
                                    ident[0:ncl + 1, 0:ncl + 1])
                rz2 = sbm.tile([P, 1], F32, tag="epi_rz2", name="rz2")
                nc.vector.reciprocal(rz2[:], pst[:, ncl:ncl + 1])
                lg = sbm.tile([P, ncl], F32, tag="epi_lg", name="lg")
                nc.vector.tensor_scalar_mul(lg[:], pst[:, 0:ncl], rz2[:])
                m = sbm.tile([P, 1], F32, tag="epi_m", name="m")
                nc.vector.tensor_reduce(m[:], lg[:], axis=AX.X, op=OP.max)
                negm = sbm.tile([P, 1], F32, tag="epi_negm", name="negm")
                nc.vector.tensor_single_scalar(negm[:], m[:], -5.0, OP.mult)
                exd = sbm.tile([P, ncl], F32, tag="epi_exd", name="exd")
                zs = sbm.tile([P, 1], F32, tag="epi_zs", name="zs")
                nc.scalar.activation(exd[:], lg[:], AF.Exp, scale=5.0,
                                     bias=negm[:, 0:1], accum_out=zs[:, 0:1])
                lnz = sbm.tile([P, 1], F32, tag="epi_lnz", name="lnz")
                nc.scalar.activation(lnz[:], zs[:], AF.Ln)
                fin = sbm.tile([P, ncl], F32, tag="epi_fin", name="fin")
                nc.vector.tensor_scalar(
                    out=fin[:], in0=lg[:], scalar1=m[:, 0:1],
                    scalar2=lnz[:, 0:1], op0=OP.subtract, op1=OP.subtract)
                nc.sync.dma_start(out_dram[_ts(c, P), :], fin[:])


# ======================= host side =======================================


def preprocess(cfg: GATConfig, x, edge_list, W1, a1, W2, a2):
    n, s = cfg.n, cfg.s
    src = np.asarray(edge_list[0]).astype(np.int64)
    tgt = np.asarray(edge_list[1]).astype(np.int64)
    key = src * n + tgt
    uniq, counts = np.unique(key, return_counts=True)
    us = (uniq // n).astype(np.int32)
    ut = (uniq % n).astype(np.int32)
    singles = counts == 1
    dups = ~singles

    maskT = np.zeros((n, n), dtype=np.float16)
    maskT[ut[singles], us[singles]] = 1.0

    row_deg = np.bincount(us, minlength=n)
    assert row_deg.min() > 0, "empty adjacency row: kernel assumes none"

    d_s, d_t, d_m = us[dups], ut[dups], counts[dups].astype(np.float32)

    x = np.asarray(x, dtype=np.float32)
    xT = np.ascontiguousarray(x.T).astype(np.float16)

    W1 = np.asarray(W1, dtype=np.float32)
    a1 = np.asarray(a1, dtype=np.float32)
    W2 = np.asarray(W2, dtype=np.float32)
    a2 = np.asarray(a2, dtype=np.float32)

    hs, nh = cfg.hs1, cfg.nhid
    # per-head columns: [W1^T (64) | va_s | va_t]; the device copies the
    # alpha results to an fp32 side tile and overwrites slot nh with ones.
    w1t = np.zeros((cfg.f_in, cfg.w1cols), np.float32)
    for h in range(cfg.heads):
        w1t[:, h * hs: h * hs + nh] = W1[h].T
        w1t[:, h * hs + nh] = W1[h].T @ a1[h, :nh]      # as
        w1t[:, h * hs + nh + 1] = W1[h].T @ a1[h, nh:]  # at
    w1t = w1t.astype(np.float16)

    ncl = cfg.nclass
    w2t = np.zeros((cfg.fcat, cfg.w2cols), np.float32)
    w2t[:, 0:ncl] = W2.T
    w2t[:, ncl] = W2.T @ a2[ncl:]       # va2_t
    w2t[:, ncl + 1] = W2.T @ a2[:ncl]   # va2_s

    sel = np.zeros((cfg.heads, cfg.heads * P), np.float32)
    for h in range(cfg.heads):
        sel[h, h * P:(h + 1) * P] = 1.0

    dup_cap = cfg.dup_rounds * P
    max_dups = 0
    for c in range(cfg.ncores):
        mine = (d_s >= c * s) & (d_s < (c + 1) * s)
        max_dups = max(max_dups, int(mine.sum()))
    assert max_dups <= dup_cap, f"{max_dups} dups > cap {dup_cap}"

    in_maps = []
    for c in range(cfg.ncores):
        blk = slice(c * s, (c + 1) * s)
        mine = np.nonzero((d_s >= c * s) & (d_s < (c + 1) * s))[0]
        k = len(mine)
        td = np.zeros((dup_cap, 1), np.int32)
        sd = np.zeros((dup_cap, 1), np.int32)
        dm = np.zeros((dup_cap, 1), np.float32)
        soh = np.zeros((dup_cap, s), np.float32)
        td[:k, 0] = d_t[mine]
        sd[:k, 0] = d_s[mine]
        dm[:k, 0] = d_m[mine]
        soh[np.arange(k), d_s[mine] - c * s] = 1.0
        in_maps.append({
            "xT": xT,
            "xTown": np.ascontiguousarray(xT[:, blk]),
            "w1t": w1t,
            "w2t": w2t,
            "maskT": np.ascontiguousarray(maskT[:, blk]),
            "tdup": td, "sdup": sd, "dmult": dm,
            "sonehot": soh.astype(ml_dtypes.bfloat16),
            "sel": sel,
        })
    return in_maps


DEBUG_IO = False


def declare_io(nc, cfg: GATConfig):
    n, s = cfg.n, cfg.s
    dup_cap = cfg.dup_rounds * P
    io = {}
    if DEBUG_IO:
        io = {
            "h1dbg": nc.dram_tensor("h1dbg", [n, cfg.w1cols], FP16, kind="ExternalOutput").ap(),
            "aldbg": nc.dram_tensor("aldbg", [n, 2 * cfg.heads], F32, kind="ExternalOutput").ap(),
            "paydbg": nc.dram_tensor("paydbg", [s, cfg.payc], FP16, kind="ExternalOutput").ap(),
            "gdbg": nc.dram_tensor("gdbg", [n, cfg.payc], FP16, kind="ExternalOutput").ap(),
            "otdbg": nc.dram_tensor("otdbg", [cfg.heads * (cfg.nhid + 1), s], BF16, kind="ExternalOutput").ap(),
            "hl2dbg": nc.dram_tensor("hl2dbg", [cfg.fcat, s], F32R, kind="ExternalOutput").ap(),
        }
    io |= {
        "xT": nc.dram_tensor("xT", [cfg.f_in, n], FP16, kind="ExternalInput").ap(),
        "xTown": nc.dram_tensor("xTown", [cfg.f_in, s], FP16, kind="ExternalInput").ap(),
        "w1t": nc.dram_tensor("w1t", [cfg.f_in, cfg.w1cols], FP16, kind="ExternalInput").ap(),
        "w2t": nc.dram_tensor("w2t", [cfg.fcat, cfg.w2cols], F32R, kind="ExternalInput").ap(),
        "maskT": nc.dram_tensor("maskT", [n, s], FP16, kind="ExternalInput").ap(),
        "tdup": nc.dram_tensor("tdup", [dup_cap, 1], I32, kind="ExternalInput").ap(),
        "sdup": nc.dram_tensor("sdup", [dup_cap, 1], I32, kind="ExternalInput").ap(),
        "dmult": nc.dram_tensor("dmult", [dup_cap, 1], F32, kind="ExternalInput").ap(),
        "sonehot": nc.dram_tensor("sonehot", [dup_cap, s], BF16, kind="ExternalInput").ap(),
        "sel": nc.dram_tensor("sel", [cfg.heads, cfg.heads * P], F32R, kind="ExternalInput").ap(),
        "out": nc.dram_tensor("out", [s, cfg.nclass], F32, kind="ExternalOutput").ap(),
    }
    return io


_ACT_PATCH_DIR = None


def install_patched_act_tables():
    """Patch exp's negative-domain buckets to compute exp(0.2*x), so one Exp
    activation evaluates exp(leaky_relu(x)). All true-exp uses in the kernel
    split pos/neg (neg scaled by 5) to stay exact under the patch."""
    global _ACT_PATCH_DIR
    import json
    import os
    import shutil
    import tempfile

    if _ACT_PATCH_DIR is not None:
        os.environ["BASS_ACT_ROOT_JSON_PATH"] = os.path.join(
            _ACT_PATCH_DIR, "act_info.json")
        return

    from neuronxcc.driver.Job import Job
    from neuronxcc.driver.jobs.support.FindActInfo import findActInfoFile

    src_json = findActInfoFile(Job.getPackageDir(), "gen3")
    src_dir = os.path.dirname(src_json)
    pwp_jsons = os.path.join(os.path.dirname(src_dir), "pwp_jsons")

    dst = tempfile.mkdtemp(prefix="act_lrelu_")
    for f in os.listdir(src_dir):
        shutil.copy(os.path.join(src_dir, f), os.path.join(dst, f))

    exp_def = json.load(open(os.path.join(pwp_jsons, "exp_400p.json")))
    neg_secs = []
    for e in exp_def["neg_exponents"]:
        for sct in e["exponent_sections"]:
            neg_secs.append(np.array(
                [sct["d0"]["int"], sct["d1"]["int"], sct["d2"]["int"],
                 sct["d3"]["int"], sct["x"]["int"]], dtype=np.uint32))

    info = json.load(open(os.path.join(dst, "act_info.json")))
    for st in info["act_func_sets"]:
        if "exp" not in st["act"]:
            continue
        path = os.path.join(dst, st["bkt_bin"])
        bkt = np.fromfile(path, dtype=np.uint32)
        view = np.lib.stride_tricks.sliding_window_view(bkt, 5)
        n_patched = 0
        for sec in neg_secs:
            m = np.where(np.all(view == sec, axis=1))[0]
            if len(m) != 1:
                continue
            i = int(m[0])
            x0 = float(sec[4:5].view(np.float32)[0])
            f = np.float32(math.exp(0.2 * x0))
            coef = np.array([f, 0.2 * f, 0.02 * f, (0.2 ** 3 / 6.0) * f],
                            dtype=np.float32)
            bkt[i:i + 4] = coef.view(np.uint32)
            n_patched += 1
        assert n_patched == len(neg_secs), (st["name"], n_patched)
        bkt.tofile(path)

    _ACT_PATCH_DIR = dst
    os.environ["BASS_ACT_ROOT_JSON_PATH"] = os.path.join(dst, "act_info.json")


def build_program(cfg: GATConfig):
    nc = bacc.Bacc("TRN2", target_bir_lowering=False, debug=False,
                   num_devices=cfg.ncores)
    io = declare_io(nc, cfg)
    with tile.TileContext(nc) as tc:
        build_gat_kernel(tc, cfg, io)
    nc.compile()
    return nc


_CACHE = {}


def kernel(x, edge_list, W1, b1, a1, W2, b2, a2, _trace=False, _tmpdir=None):
    cfg = GATConfig()
    assert np.asarray(b1).max() == 0 and np.asarray(b2).max() == 0
    in_maps = preprocess(cfg, np.asarray(x), np.asarray(edge_list),
                         np.asarray(W1), np.asarray(a1),
                         np.asarray(W2), np.asarray(a2))
    install_patched_act_tables()
    if cfg not in _CACHE:
        _CACHE[cfg] = build_program(cfg)
    nc = _CACHE[cfg]
    res = run_bass_kernel_spmd(
        nc, in_maps, core_ids=list(range(cfg.ncores)),
        trace=_trace, tmpdir=_tmpdir,
        **({"trace_cores": [0]} if _trace else {}))
    out = np.concatenate([r["out"] for r in res.results], axis=0)
    kernel._last_results = res
    return out.astype(np.float32)
